# revision 28
# baseline (speedup 1.0000x reference)
"""Trainium2 Bass kernel for nn_ConvPlus1d (dense_cnn).

Math (exact reformulation of the reference):

  The reference synthesizes per-sample conv weights
      kern[b]   = mean_L(depthwise_conv(x))          -> [B, C_IN, K]
      w_in[b]   = W_in @ kern[b]                     -> [B, C_IN, K]
      w_out[b]  = <W_out, kern[b]>                   -> [B, C_OUT]
      bias[b]   = <W_bias, kern[b]>                  -> [B, C_OUT]
      weight[b, o, c, k] = w_in[b, c, k] * w_out[b, o]
      y[b] = conv1d(x[b], weight[b], pad=1) + bias[b]

  Because weight is rank-1 across (o) x (c,k):

      y[b, o, l] = w_out[b, o] * z[b, l] + bias[b, o]
      z[b, l]    = sum_{c,k} w_in[b, c, k] * x[b, c, l + k - 1]

  so the device only has to compute the single-channel conv z and an
  outer product.  mean_L of a pad-1 depthwise conv only needs per-channel
  sums plus first/last elements, so kern (and all derived weights) are
  computed on the host in float64 from (S, E, F) and shipped down as tiny
  per-sample tensors.

Device program (per core, 4 samples, fp16 data / fp32 PSUM):
  x is shipped even/odd interleaved: xeo[0:64, j] = x[:, 2(j-1)],
  xeo[64:128, j] = x[:, 2(j-1)+1].  For m-tile columns:
      z_odd [m] = win0.xe[m] + win1.xo[m] + win2.xe[m+1]   (l = 2m+1)
      z_even[m] = win1.xe[m] + win2.xo[m] + win0.xo[m-1]   (l = 2m)
  Each parity is 2 matmuls: one 128-deep packed pass + one 64-deep
  correction pass.  The stationary matrices have 128 IDENTICAL columns
  (v (x) ones), so the matmul materializes z replicated across all 128
  PSUM partitions -- the outer product then costs a single per-tile
  tensor_scalar (DVE) / activation (ACT): out = z * w_out[o] + bias[o],
  evicting PSUM straight to fp16 SBUF.

Sharding: batch 32 -> 8 cores x 4 samples.  Host interleaves the two
parity planes and widens fp16 -> fp32 on gather.
"""

import sys

import ml_dtypes
import numpy as np

sys.path.insert(0, "/opt/trn_rl_repo")

import concourse.bacc as bacc  # noqa: E402
import concourse.tile as tile  # noqa: E402
from concourse import mybir  # noqa: E402
from concourse.bass_utils import run_bass_kernel_spmd  # noqa: E402

B, C_IN, C_OUT, K, L = 32, 64, 128, 3, 8192
N_CORES = 8
BS = B // N_CORES          # samples per core
M = L // 2                 # columns per parity plane
NT = 512                   # matmul moving-dim tile (one PSUM bank of fp32)
NJ = M // 2 + 2            # columns per x chunk (2 chunks, 2-col overlap)

F8 = mybir.dt.float8e4
F16 = mybir.dt.float16
F32 = mybir.dt.float32


def _host_synth(x, W_kernel, W_in, W_out, W_bias):
    """Per-sample weight synthesis in float64 (exact)."""
    xd = x.astype(np.float64)
    S = xd.sum(axis=2)                                       # [B, C]
    E = xd[:, :, -1]
    F = xd[:, :, 0]
    sig = np.stack([S - E, S, S - F], axis=2)                # [B, C, 3(tap)]

    Wk3 = W_kernel.reshape(C_IN, K, K).astype(np.float64)    # [c, j, tap]
    kern = np.einsum("cjt,bct->bcj", Wk3, sig) / L           # [B, C, K]

    Win = W_in[:, :, 0].astype(np.float64)                   # [c', c]
    w_in = np.einsum("pc,bck->bpk", Win, kern)               # [B, C, K]
    w_out = np.einsum("ock,bck->bo", W_out.astype(np.float64), kern)
    bias = np.einsum("ock,bck->bo", W_bias.astype(np.float64), kern)
    return w_in, w_out, bias


_CACHE = {}


def _build_module():
    if "nc" in _CACHE:
        return _CACHE["nc"]
    nc = bacc.Bacc("TRN2", target_bir_lowering=False, debug=False)

    x_d = nc.dram_tensor("x", [BS, 2, 128, NJ], F8,
                         kind="ExternalInput").ap()
    # all samples' weights in one tensor each: one DMA apiece.  The
    # stationary matrices have 128 identical columns, stored once and
    # broadcast via a stride-0 free dim in the lhsT AP.
    wz_d = nc.dram_tensor("wz", [128, BS, 3], F8,
                          kind="ExternalInput").ap()
    wv_d = nc.dram_tensor("wv", [128, BS, 2], F32,
                          kind="ExternalInput").ap()
    y_d = nc.dram_tensor("y", [BS, 2, C_OUT, M], F16,
                         kind="ExternalOutput").ap()

    TPC = NJ - 2           # m-columns produced per chunk (2048)
    with tile.TileContext(nc) as tc:
        with (
            tc.tile_pool(name="consts", bufs=1) as consts,
            tc.tile_pool(name="xp", bufs=2 * BS) as xp,
            tc.tile_pool(name="yp", bufs=4) as yp,
            tc.tile_pool(name="ps", bufs=4, space="PSUM") as ps,
        ):
            # prefetch: first half-chunk of sample 0 first (gates the first
            # matmul), then all weights in one DMA, then remaining chunks
            NH = NJ // 2 + 1           # half-chunk cols, 2-col overlap
            xc00a = xp.tile([128, NH], F8, tag="xh")
            nc.sync.dma_start(xc00a[:], x_d[0][0][:, 0:NH])
            wza = consts.tile([128, BS, 3], F8, tag="wz")
            wva = consts.tile([128, BS, 2], F32, tag="wv")
            nc.sync.dma_start(wza[:], wz_d)
            nc.sync.dma_start(wva[:], wv_d)
            xc00b = xp.tile([128, NH], F8, tag="xh")
            nc.sync.dma_start(xc00b[:], x_d[0][0][:, NJ - NH:NJ])
            xcs = [(xc00a, xc00b)]
            for s in range(BS):
                for c in range(2):
                    if s == 0 and c == 0:
                        continue
                    xc = xp.tile([128, NJ], F8, tag="xc")
                    nc.sync.dma_start(xc[:], x_d[s][c])
                    xcs.append((xc, xc))

            for s in range(BS):
                wz_s = [wza[:, s, k:k + 1].broadcast_to([128, C_OUT])
                        for k in range(3)]
                wz_ct = wza[0:64, s, 2:3].broadcast_to([64, C_OUT])
                wz_cb = wza[64:128, s, 2:3].broadcast_to([64, C_OUT])
                w_out_ap = wva[:, s, 0:1]
                bias_ap = wva[:, s, 1:2]
                for c in range(2):
                    xca, xcb = xcs[2 * s + c]
                    split = xca is not xcb
                    yo = yp.tile([C_OUT, TPC], F16, tag="yo")
                    ye = yp.tile([C_OUT, TPC], F16, tag="ye")
                    for th in range(2):
                        # 2-bank PSUM tiles; one eviction per 1024 cols
                        xc = xcb if (split and th) else xca
                        off = (NJ - NH) if (split and th) else 0
                        zo = ps.tile([C_OUT, 2 * NT], F32, tag="z2")
                        ze = ps.tile([C_OUT, 2 * NT], F32, tag="z2")
                        # all full-K packed passes first, then the K=64
                        # corrections: adjacent corrections auto-derive PE
                        # row tile_positions (0,0)/(64,0) and overlap, and
                        # grouping them pays the 128<->64-row geometry
                        # switch once per 2 tiles instead of twice
                        for q in range(2):
                            jm = 2 * NT * th + NT * q + 1 - off
                            qs = slice(NT * q, NT * (q + 1))
                            nc.tensor.matmul(zo[:, qs], wz_s[0],
                                             xc[:, jm:jm + NT],
                                             start=True, stop=False)
                            nc.tensor.matmul(ze[:, qs], wz_s[1],
                                             xc[:, jm:jm + NT],
                                             start=True, stop=False)
                        for q in range(2):
                            jm = 2 * NT * th + NT * q + 1 - off
                            qs = slice(NT * q, NT * (q + 1))
                            nc.tensor.matmul(zo[:, qs], wz_ct,
                                             xc[0:64, jm + 1:jm + 1 + NT],
                                             start=False, stop=True)
                            nc.tensor.matmul(ze[:, qs], wz_cb,
                                             xc[64:128, jm - 1:jm - 1 + NT],
                                             start=False, stop=True)
                        sl = slice(2 * NT * th, 2 * NT * (th + 1))
                        nc.vector.tensor_scalar(
                            out=yo[:, sl], in0=zo[:],
                            scalar1=w_out_ap, scalar2=bias_ap,
                            op0=mybir.AluOpType.mult,
                            op1=mybir.AluOpType.add)
                        nc.scalar.activation(
                            ye[:, sl], ze[:],
                            mybir.ActivationFunctionType.Identity,
                            bias=bias_ap, scale=w_out_ap)
                        # store each 1024-col half as soon as it's evicted:
                        # keeps DMA busy and shortens the end-of-kernel tail
                        dsl = slice(TPC * c + 2 * NT * th,
                                    TPC * c + 2 * NT * (th + 1))
                        nc.scalar.dma_start(y_d[s][0][:, dsl], ye[:, sl])
                        nc.sync.dma_start(y_d[s][1][:, dsl], yo[:, sl])

    nc.compile()
    _CACHE["nc"] = nc
    return nc


def kernel(x, W_kernel, W_in, W_out, W_bias):
    x = np.asarray(x, dtype=np.float32)
    w_in, w_out, bias = _host_synth(
        x, np.asarray(W_kernel, np.float32), np.asarray(W_in, np.float32),
        np.asarray(W_out, np.float32), np.asarray(W_bias, np.float32))

    # rescale w_in per sample into fp8's sweet range; fold the scale
    # into w_out (exact)
    wscale = np.abs(w_in).max(axis=(1, 2), keepdims=True)
    w_in = w_in / wscale * 8.0
    w_out = w_out * wscale[:, :, 0] / 8.0

    # even/odd interleaved x: xeo[:, 0:64, j] = x[:, :, 2(j-1)],
    # xeo[:, 64:128, j] = x[:, :, 2(j-1)+1], zero-padded at both ends,
    # split into 2 chunks with a 2-column overlap.
    x8 = x.astype(ml_dtypes.float8_e4m3)
    xeo = np.zeros((B, 128, M + 2), ml_dtypes.float8_e4m3)
    xeo[:, 0:64, 1:M + 1] = x8[:, :, 0::2]
    xeo[:, 64:128, 1:M + 1] = x8[:, :, 1::2]
    xch = np.stack([xeo[:, :, 0:NJ], xeo[:, :, M + 2 - NJ:M + 2]], axis=1)

    # stationary matrices: 128 identical columns of the packed w_in taps
    w0 = w_in[:, :, 0]
    w1 = w_in[:, :, 1]
    w2 = w_in[:, :, 2]
    v = np.stack([
        np.concatenate([w0, w1], axis=1),    # packed pass, odd parity
        np.concatenate([w1, w2], axis=1),    # packed pass, even parity
        np.concatenate([w2, w0], axis=1),    # corrections (top/bottom half)
    ], axis=1)                               # [B, 3, 128]
    # [128, B, 3]: one stationary column per (sample, tap-pack)
    wz = np.ascontiguousarray(v.astype(ml_dtypes.float8_e4m3).transpose(2, 0, 1))
    wv = np.stack([w_out.T, bias.T], axis=2).astype(np.float32)  # [128,B,2]

    nc = _build_module()
    in_maps = [
        {"x": xch[c * BS:(c + 1) * BS],
         "wz": np.ascontiguousarray(wz[:, c * BS:(c + 1) * BS]),
         "wv": np.ascontiguousarray(wv[:, c * BS:(c + 1) * BS])}
        for c in range(N_CORES)
    ]
    res = run_bass_kernel_spmd(nc, in_maps, core_ids=list(range(N_CORES)))
    global LAST_RESULT
    LAST_RESULT = res

    yp = np.concatenate([r["y"] for r in res.results], axis=0)  # [B,2,O,M]
    y = np.empty((B, C_OUT, L), np.float32)
    y[:, :, 0::2] = yp[:, 0].astype(np.float32)
    y[:, :, 1::2] = yp[:, 1].astype(np.float32)
    return y


LAST_RESULT = None


# revision 32
# speedup vs baseline: 1.0352x; 1.0352x over previous
"""Trainium2 Bass kernel for nn_ConvPlus1d (dense_cnn).

Math (exact reformulation of the reference):

  The reference synthesizes per-sample conv weights
      kern[b]   = mean_L(depthwise_conv(x))          -> [B, C_IN, K]
      w_in[b]   = W_in @ kern[b]                     -> [B, C_IN, K]
      w_out[b]  = <W_out, kern[b]>                   -> [B, C_OUT]
      bias[b]   = <W_bias, kern[b]>                  -> [B, C_OUT]
      weight[b, o, c, k] = w_in[b, c, k] * w_out[b, o]
      y[b] = conv1d(x[b], weight[b], pad=1) + bias[b]

  Because weight is rank-1 across (o) x (c,k):

      y[b, o, l] = w_out[b, o] * z[b, l] + bias[b, o]
      z[b, l]    = sum_{c,k} w_in[b, c, k] * x[b, c, l + k - 1]

  so the device only has to compute the single-channel conv z and an
  outer product.  mean_L of a pad-1 depthwise conv only needs per-channel
  sums plus first/last elements, so kern (and all derived weights) are
  computed on the host in float64 from (S, E, F) and shipped down as tiny
  per-sample tensors.

Device program (per core, 4 samples, fp16 data / fp32 PSUM):
  x is shipped even/odd interleaved: xeo[0:64, j] = x[:, 2(j-1)],
  xeo[64:128, j] = x[:, 2(j-1)+1].  For m-tile columns:
      z_odd [m] = win0.xe[m] + win1.xo[m] + win2.xe[m+1]   (l = 2m+1)
      z_even[m] = win1.xe[m] + win2.xo[m] + win0.xo[m-1]   (l = 2m)
  Each parity is 2 matmuls: one 128-deep packed pass + one 64-deep
  correction pass.  The stationary matrices have 128 IDENTICAL columns
  (v (x) ones), so the matmul materializes z replicated across all 128
  PSUM partitions -- the outer product then costs a single per-tile
  tensor_scalar (DVE) / activation (ACT): out = z * w_out[o] + bias[o],
  evicting PSUM straight to fp16 SBUF.

Sharding: batch 32 -> 8 cores x 4 samples.  Host interleaves the two
parity planes and widens fp16 -> fp32 on gather.
"""

import sys

import ml_dtypes
import numpy as np

sys.path.insert(0, "/opt/trn_rl_repo")

import concourse.bacc as bacc  # noqa: E402
import concourse.tile as tile  # noqa: E402
from concourse import mybir  # noqa: E402
from concourse.bass_utils import run_bass_kernel_spmd  # noqa: E402

B, C_IN, C_OUT, K, L = 32, 64, 128, 3, 8192
N_CORES = 8
BS = B // N_CORES          # samples per core
M = L // 2                 # columns per parity plane
NT = 512                   # matmul moving-dim tile (one PSUM bank of fp32)
NJ = M // 2 + 2            # columns per x chunk (2 chunks, 2-col overlap)

F8 = mybir.dt.float8e4
F16 = mybir.dt.float16
F32 = mybir.dt.float32


def _host_synth(x, W_kernel, W_in, W_out, W_bias):
    """Per-sample weight synthesis in float64 (exact)."""
    xd = x.astype(np.float64)
    S = xd.sum(axis=2)                                       # [B, C]
    E = xd[:, :, -1]
    F = xd[:, :, 0]
    sig = np.stack([S - E, S, S - F], axis=2)                # [B, C, 3(tap)]

    Wk3 = W_kernel.reshape(C_IN, K, K).astype(np.float64)    # [c, j, tap]
    kern = np.einsum("cjt,bct->bcj", Wk3, sig) / L           # [B, C, K]

    Win = W_in[:, :, 0].astype(np.float64)                   # [c', c]
    w_in = np.einsum("pc,bck->bpk", Win, kern)               # [B, C, K]
    w_out = np.einsum("ock,bck->bo", W_out.astype(np.float64), kern)
    bias = np.einsum("ock,bck->bo", W_bias.astype(np.float64), kern)
    return w_in, w_out, bias


_CACHE = {}


def _build_module():
    if "nc" in _CACHE:
        return _CACHE["nc"]
    nc = bacc.Bacc("TRN2", target_bir_lowering=False, debug=False)

    NJF = M + 2            # full per-sample column count (4098)
    x_d = nc.dram_tensor("x", [128, BS, NJF], F8,
                         kind="ExternalInput").ap()
    # all samples' weights in one tensor each: one DMA apiece.  The
    # stationary matrices have 128 identical columns, stored once and
    # broadcast via a stride-0 free dim in the lhsT AP.
    wz_d = nc.dram_tensor("wz", [128, BS, 3], F8,
                          kind="ExternalInput").ap()
    wv_d = nc.dram_tensor("wv", [128, BS, 2], F32,
                          kind="ExternalInput").ap()
    y_d = nc.dram_tensor("y", [BS, 2, C_OUT, M], F16,
                         kind="ExternalOutput").ap()

    NHA = 4 * NT + 2       # sample-0 first-half cols (j 0..2049)
    with tile.TileContext(nc) as tc:
        with (
            tc.tile_pool(name="consts", bufs=1) as consts,
            tc.tile_pool(name="xp", bufs=1) as xp,
            tc.tile_pool(name="yp", bufs=4) as yp,
            tc.tile_pool(name="ps", bufs=4, space="PSUM") as ps,
        ):
            # prefetch: sample-0 first half gates the first matmul; then
            # weights; then the rest of x in two big DMAs (12KB runs)
            x0a = xp.tile([128, NHA], F8, tag="x0a")
            nc.sync.dma_start(x0a[:], x_d[:, 0, 0:NHA])
            wza = consts.tile([128, BS, 3], F8, tag="wz")
            wva = consts.tile([128, BS, 2], F32, tag="wv")
            nc.sync.dma_start(wza[:], wz_d)
            nc.sync.dma_start(wva[:], wv_d)
            x0b = xp.tile([128, NJF - 2 * 2 * NT], F8, tag="x0b")
            nc.sync.dma_start(x0b[:], x_d[:, 0, 2 * 2 * NT:NJF])
            xr = xp.tile([128, BS - 1, NJF], F8, tag="xr")
            nc.sync.dma_start(xr[:], x_d[:, 1:BS, :])

            for s in range(BS):
                wz_s = [wza[:, s, k:k + 1].broadcast_to([128, C_OUT])
                        for k in range(3)]
                wz_ct = wza[0:64, s, 2:3].broadcast_to([64, C_OUT])
                wz_cb = wza[64:128, s, 2:3].broadcast_to([64, C_OUT])
                w_out_ap = wva[:, s, 0:1]
                bias_ap = wva[:, s, 1:2]
                yo = yp.tile([C_OUT, M], F16, tag="yo")
                ye = yp.tile([C_OUT, M], F16, tag="ye")
                for th in range(4):
                    if s == 0:
                        xc = x0a if th < 2 else x0b
                        off = 0 if th < 2 else 2 * 2 * NT
                    else:
                        xc = xr[:, s - 1, :]
                        off = 0
                    zo = ps.tile([C_OUT, 2 * NT], F32, tag="z2")
                    ze = ps.tile([C_OUT, 2 * NT], F32, tag="z2")
                    # all full-K packed passes first, then the K=64
                    # corrections: adjacent corrections auto-derive PE row
                    # tile_positions (0,0)/(64,0) and overlap, and grouping
                    # them pays the 128<->64-row geometry switch once per
                    # 2 tiles instead of twice
                    for q in range(2):
                        jm = 2 * NT * th + NT * q + 1 - off
                        qs = slice(NT * q, NT * (q + 1))
                        nc.tensor.matmul(zo[:, qs], wz_s[0],
                                         xc[:, jm:jm + NT],
                                         start=True, stop=False)
                        nc.tensor.matmul(ze[:, qs], wz_s[1],
                                         xc[:, jm:jm + NT],
                                         start=True, stop=False)
                    for q in range(2):
                        jm = 2 * NT * th + NT * q + 1 - off
                        qs = slice(NT * q, NT * (q + 1))
                        nc.tensor.matmul(zo[:, qs], wz_ct,
                                         xc[0:64, jm + 1:jm + 1 + NT],
                                         start=False, stop=True)
                        nc.tensor.matmul(ze[:, qs], wz_cb,
                                         xc[64:128, jm - 1:jm - 1 + NT],
                                         start=False, stop=True)
                    sl = slice(2 * NT * th, 2 * NT * (th + 1))
                    nc.vector.tensor_scalar(
                        out=yo[:, sl], in0=zo[:],
                        scalar1=w_out_ap, scalar2=bias_ap,
                        op0=mybir.AluOpType.mult,
                        op1=mybir.AluOpType.add)
                    nc.scalar.activation(
                        ye[:, sl], ze[:],
                        mybir.ActivationFunctionType.Identity,
                        bias=bias_ap, scale=w_out_ap)
                    if th % 2 == 1:
                        # store per 2048 evicted cols, split across queues
                        dsl = slice(2 * NT * (th - 1), 2 * NT * (th + 1))
                        nc.scalar.dma_start(y_d[s][0][:, dsl], ye[:, dsl])
                        nc.sync.dma_start(y_d[s][1][:, dsl], yo[:, dsl])

    nc.compile()
    _CACHE["nc"] = nc
    return nc


def kernel(x, W_kernel, W_in, W_out, W_bias):
    x = np.asarray(x, dtype=np.float32)
    w_in, w_out, bias = _host_synth(
        x, np.asarray(W_kernel, np.float32), np.asarray(W_in, np.float32),
        np.asarray(W_out, np.float32), np.asarray(W_bias, np.float32))

    # rescale w_in per sample into fp8's sweet range; fold the scale
    # into w_out (exact)
    wscale = np.abs(w_in).max(axis=(1, 2), keepdims=True)
    w_in = w_in / wscale * 8.0
    w_out = w_out * wscale[:, :, 0] / 8.0

    # even/odd interleaved x: xeo[:, 0:64, j] = x[:, :, 2(j-1)],
    # xeo[:, 64:128, j] = x[:, :, 2(j-1)+1], zero-padded at both ends,
    # split into 2 chunks with a 2-column overlap.
    x8 = x.astype(ml_dtypes.float8_e4m3)
    xeo = np.zeros((128, B, M + 2), ml_dtypes.float8_e4m3)
    xeo[0:64, :, 1:M + 1] = x8[:, :, 0::2].transpose(1, 0, 2)
    xeo[64:128, :, 1:M + 1] = x8[:, :, 1::2].transpose(1, 0, 2)

    # stationary matrices: 128 identical columns of the packed w_in taps
    w0 = w_in[:, :, 0]
    w1 = w_in[:, :, 1]
    w2 = w_in[:, :, 2]
    v = np.stack([
        np.concatenate([w0, w1], axis=1),    # packed pass, odd parity
        np.concatenate([w1, w2], axis=1),    # packed pass, even parity
        np.concatenate([w2, w0], axis=1),    # corrections (top/bottom half)
    ], axis=1)                               # [B, 3, 128]
    # [128, B, 3]: one stationary column per (sample, tap-pack)
    wz = np.ascontiguousarray(v.astype(ml_dtypes.float8_e4m3).transpose(2, 0, 1))
    wv = np.stack([w_out.T, bias.T], axis=2).astype(np.float32)  # [128,B,2]

    nc = _build_module()
    in_maps = [
        {"x": np.ascontiguousarray(xeo[:, c * BS:(c + 1) * BS]),
         "wz": np.ascontiguousarray(wz[:, c * BS:(c + 1) * BS]),
         "wv": np.ascontiguousarray(wv[:, c * BS:(c + 1) * BS])}
        for c in range(N_CORES)
    ]
    res = run_bass_kernel_spmd(nc, in_maps, core_ids=list(range(N_CORES)))
    global LAST_RESULT
    LAST_RESULT = res

    yp = np.concatenate([r["y"] for r in res.results], axis=0)  # [B,2,O,M]
    y = np.empty((B, C_OUT, L), np.float32)
    y[:, :, 0::2] = yp[:, 0].astype(np.float32)
    y[:, :, 1::2] = yp[:, 1].astype(np.float32)
    return y


LAST_RESULT = None


# revision 37
# speedup vs baseline: 1.0426x; 1.0071x over previous
"""Trainium2 Bass kernel for nn_ConvPlus1d (dense_cnn).

Math (exact reformulation of the reference):

  The reference synthesizes per-sample conv weights
      kern[b]   = mean_L(depthwise_conv(x))          -> [B, C_IN, K]
      w_in[b]   = W_in @ kern[b]                     -> [B, C_IN, K]
      w_out[b]  = <W_out, kern[b]>                   -> [B, C_OUT]
      bias[b]   = <W_bias, kern[b]>                  -> [B, C_OUT]
      weight[b, o, c, k] = w_in[b, c, k] * w_out[b, o]
      y[b] = conv1d(x[b], weight[b], pad=1) + bias[b]

  Because weight is rank-1 across (o) x (c,k):

      y[b, o, l] = w_out[b, o] * z[b, l] + bias[b, o]
      z[b, l]    = sum_{c,k} w_in[b, c, k] * x[b, c, l + k - 1]

  so the device only has to compute the single-channel conv z and an
  outer product.  mean_L of a pad-1 depthwise conv only needs per-channel
  sums plus first/last elements, so kern (and all derived weights) are
  computed on the host in float64 from (S, E, F) and shipped down as tiny
  per-sample tensors.

Device program (per core, 4 samples, fp16 data / fp32 PSUM):
  x is shipped even/odd interleaved: xeo[0:64, j] = x[:, 2(j-1)],
  xeo[64:128, j] = x[:, 2(j-1)+1].  For m-tile columns:
      z_odd [m] = win0.xe[m] + win1.xo[m] + win2.xe[m+1]   (l = 2m+1)
      z_even[m] = win1.xe[m] + win2.xo[m] + win0.xo[m-1]   (l = 2m)
  Each parity is 2 matmuls: one 128-deep packed pass + one 64-deep
  correction pass.  The stationary matrices have 128 IDENTICAL columns
  (v (x) ones), so the matmul materializes z replicated across all 128
  PSUM partitions -- the outer product then costs a single per-tile
  tensor_scalar (DVE) / activation (ACT): out = z * w_out[o] + bias[o],
  evicting PSUM straight to fp16 SBUF.

Sharding: batch 32 -> 8 cores x 4 samples.  Host interleaves the two
parity planes and widens fp16 -> fp32 on gather.
"""

import sys

import ml_dtypes
import numpy as np

sys.path.insert(0, "/opt/trn_rl_repo")

import concourse.bacc as bacc  # noqa: E402
import concourse.tile as tile  # noqa: E402
from concourse import mybir  # noqa: E402
from concourse.bass_utils import run_bass_kernel_spmd  # noqa: E402

B, C_IN, C_OUT, K, L = 32, 64, 128, 3, 8192
N_CORES = 8
BS = B // N_CORES          # samples per core
M = L // 2                 # columns per parity plane
NT = 512                   # matmul moving-dim tile (one PSUM bank of fp32)
NJ = M // 2 + 2            # columns per x chunk (2 chunks, 2-col overlap)

F8 = mybir.dt.float8e4
F16 = mybir.dt.float16
F32 = mybir.dt.float32


def _host_synth(x, W_kernel, W_in, W_out, W_bias):
    """Per-sample weight synthesis in float64 (exact)."""
    xd = x.astype(np.float64)
    S = xd.sum(axis=2)                                       # [B, C]
    E = xd[:, :, -1]
    F = xd[:, :, 0]
    sig = np.stack([S - E, S, S - F], axis=2)                # [B, C, 3(tap)]

    Wk3 = W_kernel.reshape(C_IN, K, K).astype(np.float64)    # [c, j, tap]
    kern = np.einsum("cjt,bct->bcj", Wk3, sig) / L           # [B, C, K]

    Win = W_in[:, :, 0].astype(np.float64)                   # [c', c]
    w_in = np.einsum("pc,bck->bpk", Win, kern)               # [B, C, K]
    w_out = np.einsum("ock,bck->bo", W_out.astype(np.float64), kern)
    bias = np.einsum("ock,bck->bo", W_bias.astype(np.float64), kern)
    return w_in, w_out, bias


_CACHE = {}


def _build_module():
    if "nc" in _CACHE:
        return _CACHE["nc"]
    nc = bacc.Bacc("TRN2", target_bir_lowering=False, debug=False)

    NJF = M + 2            # full per-sample column count (4098)
    x_d = nc.dram_tensor("x", [128, BS, NJF], F8,
                         kind="ExternalInput").ap()
    # all samples' weights in one tensor each: one DMA apiece.  The
    # stationary matrices have 128 identical columns, stored once and
    # broadcast via a stride-0 free dim in the lhsT AP.
    wz_d = nc.dram_tensor("wz", [128, BS, 3], F8,
                          kind="ExternalInput").ap()
    wv_d = nc.dram_tensor("wv", [128, BS, 2], F32,
                          kind="ExternalInput").ap()
    y_d = nc.dram_tensor("y", [BS, 2, C_OUT, M], F16,
                         kind="ExternalOutput").ap()

    NHA = 2 * NT + 2       # sample-0 first-tile cols (j 0..1025)
    with tile.TileContext(nc) as tc:
        with (
            tc.tile_pool(name="consts", bufs=1) as consts,
            tc.tile_pool(name="xp", bufs=1) as xp,
            tc.tile_pool(name="yp", bufs=4) as yp,
            tc.tile_pool(name="ps", bufs=4, space="PSUM") as ps,
        ):
            # prefetch: sample-0 first quarter gates the first matmul
            # (weights go on the idle vector queue, concurrently); then
            # the rest of x in two big DMAs (12KB runs)
            x0a = xp.tile([128, NHA], F8, tag="x0a")
            nc.sync.dma_start(x0a[:], x_d[:, 0, 0:NHA])
            wza = consts.tile([128, BS, 3], F8, tag="wz")
            wva = consts.tile([128, BS, 2], F32, tag="wv")
            nc.scalar.dma_start(wza[:], wz_d)
            nc.scalar.dma_start(wva[:], wv_d)
            x0b = xp.tile([128, NJF - 2 * NT], F8, tag="x0b")
            nc.sync.dma_start(x0b[:], x_d[:, 0, 2 * NT:NJF])
            xr = xp.tile([128, BS - 1, NJF], F8, tag="xr")
            nc.sync.dma_start(xr[:], x_d[:, 1:BS, :])

            for s in range(BS):
                wz_s = [wza[:, s, k:k + 1].broadcast_to([128, C_OUT])
                        for k in range(3)]
                wz_ct = wza[0:64, s, 2:3].broadcast_to([64, C_OUT])
                wz_cb = wza[64:128, s, 2:3].broadcast_to([64, C_OUT])
                w_out_ap = wva[:, s, 0:1]
                bias_ap = wva[:, s, 1:2]
                yo = yp.tile([C_OUT, M], F16, tag="yo")
                ye = yp.tile([C_OUT, M], F16, tag="ye")
                for th in range(4):
                    if s == 0:
                        xc = x0a if th < 1 else x0b
                        off = 0 if th < 1 else 2 * NT
                    else:
                        xc = xr[:, s - 1, :]
                        off = 0
                    zo = ps.tile([C_OUT, 2 * NT], F32, tag="z2")
                    ze = ps.tile([C_OUT, 2 * NT], F32, tag="z2")
                    # all full-K packed passes first, then the K=64
                    # corrections: adjacent corrections auto-derive PE row
                    # tile_positions (0,0)/(64,0) and overlap, and grouping
                    # them pays the 128<->64-row geometry switch once per
                    # 2 tiles instead of twice
                    for q in range(2):
                        jm = 2 * NT * th + NT * q + 1 - off
                        qs = slice(NT * q, NT * (q + 1))
                        nc.tensor.matmul(zo[:, qs], wz_s[0],
                                         xc[:, jm:jm + NT],
                                         start=True, stop=False)
                        nc.tensor.matmul(ze[:, qs], wz_s[1],
                                         xc[:, jm:jm + NT],
                                         start=True, stop=False)
                    for q in range(2):
                        jm = 2 * NT * th + NT * q + 1 - off
                        qs = slice(NT * q, NT * (q + 1))
                        nc.tensor.matmul(zo[:, qs], wz_ct,
                                         xc[0:64, jm + 1:jm + 1 + NT],
                                         start=False, stop=True)
                        nc.tensor.matmul(ze[:, qs], wz_cb,
                                         xc[64:128, jm - 1:jm - 1 + NT],
                                         start=False, stop=True)
                    sl = slice(2 * NT * th, 2 * NT * (th + 1))
                    nc.vector.tensor_scalar(
                        out=yo[:, sl], in0=zo[:],
                        scalar1=w_out_ap, scalar2=bias_ap,
                        op0=mybir.AluOpType.mult,
                        op1=mybir.AluOpType.add)
                    nc.scalar.activation(
                        ye[:, sl], ze[:],
                        mybir.ActivationFunctionType.Identity,
                        bias=bias_ap, scale=w_out_ap)
                    if s == BS - 1 and th >= 2:
                        # finer stores at the very end shorten the tail
                        nc.scalar.dma_start(y_d[s][0][:, sl], ye[:, sl])
                        nc.sync.dma_start(y_d[s][1][:, sl], yo[:, sl])
                    elif th % 2 == 1:
                        # store per 2048 evicted cols, split across queues
                        dsl = slice(2 * NT * (th - 1), 2 * NT * (th + 1))
                        nc.scalar.dma_start(y_d[s][0][:, dsl], ye[:, dsl])
                        nc.sync.dma_start(y_d[s][1][:, dsl], yo[:, dsl])

    nc.compile()
    _CACHE["nc"] = nc
    return nc


def kernel(x, W_kernel, W_in, W_out, W_bias):
    x = np.asarray(x, dtype=np.float32)
    w_in, w_out, bias = _host_synth(
        x, np.asarray(W_kernel, np.float32), np.asarray(W_in, np.float32),
        np.asarray(W_out, np.float32), np.asarray(W_bias, np.float32))

    # rescale w_in per sample into fp8's sweet range; fold the scale
    # into w_out (exact)
    wscale = np.abs(w_in).max(axis=(1, 2), keepdims=True)
    w_in = w_in / wscale * 8.0
    w_out = w_out * wscale[:, :, 0] / 8.0

    # even/odd interleaved x: xeo[:, 0:64, j] = x[:, :, 2(j-1)],
    # xeo[:, 64:128, j] = x[:, :, 2(j-1)+1], zero-padded at both ends,
    # split into 2 chunks with a 2-column overlap.
    x8 = x.astype(ml_dtypes.float8_e4m3)
    xeo = np.zeros((128, B, M + 2), ml_dtypes.float8_e4m3)
    xeo[0:64, :, 1:M + 1] = x8[:, :, 0::2].transpose(1, 0, 2)
    xeo[64:128, :, 1:M + 1] = x8[:, :, 1::2].transpose(1, 0, 2)

    # stationary matrices: 128 identical columns of the packed w_in taps
    w0 = w_in[:, :, 0]
    w1 = w_in[:, :, 1]
    w2 = w_in[:, :, 2]
    v = np.stack([
        np.concatenate([w0, w1], axis=1),    # packed pass, odd parity
        np.concatenate([w1, w2], axis=1),    # packed pass, even parity
        np.concatenate([w2, w0], axis=1),    # corrections (top/bottom half)
    ], axis=1)                               # [B, 3, 128]
    # [128, B, 3]: one stationary column per (sample, tap-pack)
    wz = np.ascontiguousarray(v.astype(ml_dtypes.float8_e4m3).transpose(2, 0, 1))
    wv = np.stack([w_out.T, bias.T], axis=2).astype(np.float32)  # [128,B,2]

    nc = _build_module()
    in_maps = [
        {"x": np.ascontiguousarray(xeo[:, c * BS:(c + 1) * BS]),
         "wz": np.ascontiguousarray(wz[:, c * BS:(c + 1) * BS]),
         "wv": np.ascontiguousarray(wv[:, c * BS:(c + 1) * BS])}
        for c in range(N_CORES)
    ]
    res = run_bass_kernel_spmd(nc, in_maps, core_ids=list(range(N_CORES)))
    global LAST_RESULT
    LAST_RESULT = res

    yp = np.concatenate([r["y"] for r in res.results], axis=0)  # [B,2,O,M]
    y = np.empty((B, C_OUT, L), np.float32)
    y[:, :, 0::2] = yp[:, 0].astype(np.float32)
    y[:, :, 1::2] = yp[:, 1].astype(np.float32)
    return y


LAST_RESULT = None
